# revision 1
# baseline (speedup 1.0000x reference)
"""Trainium2 Bass kernel for nn_EquivariantHardAlignmentModel.

8 NeuronCores, SPMD (identical program, per-core data):
  - The three 512-step LSTM recurrences (enc fwd, enc bwd, dec) are
    replicated on every core with the full batch of 32: per-step PE cost is
    set by streaming Whh^T and is batch-independent, so replication avoids
    all cross-core traffic.  Phase A = fwd + first half of bwd, phase B =
    dec (as two 16-batch chains) + second half of bwd, so >=2 independent
    chains always pipeline the engines.
  - The G-stack (embed/conv/logits/Z), ys gathers, bilinear alignment and
    loss tail are data-parallel: each core does 4 of 32 batch rows.  Inputs
    are batch-permuted per core so its rows are always rows 0..3 -> one
    shared program.
  - p[b,j] = log(sum_i exp(lys+eij-lnZ)) - log(sum_i exp(eij)) via
    PSUM-accumulated matmuls + ACT Exp(accum_out).  Host sums & negates.
"""

import os
import sys

sys.path.insert(0, "/opt/trn_rl_repo")

import numpy as np
import ml_dtypes

import concourse.bass as bass
import concourse.mybir as mybir
import concourse.tile as tile
from concourse import bacc
from concourse.bass_utils import run_bass_kernel_spmd
from concourse.masks import make_identity

BF = mybir.dt.bfloat16
F32 = mybir.dt.float32
AF = mybir.ActivationFunctionType

B, NE, ND = 32, 512, 512
V = 2000
H, F, KW, PG = 256, 256, 5, 4
EE, ED = 128, 128
NCORES, BPC = 8, 4

# torch gate blocks i,f,g,o (256 each) -> reordered [i f o g]
GATE_PERM = np.concatenate(
    [np.arange(0, 512), np.arange(768, 1024), np.arange(512, 768)]
)


def _bf(x):
    return np.ascontiguousarray(x.astype(ml_dtypes.bfloat16))


def _wrap16(flat):
    """index list -> (128, n/16) int16, dma_gather wrapped + 8x replicated."""
    flat = np.asarray(flat).reshape(-1)
    assert flat.size % 16 == 0
    w = flat.reshape(-1, 16).T.astype(np.int16)  # (16, n/16)
    return np.ascontiguousarray(np.tile(w, (8, 1)))


# ---------------------------------------------------------------------------
# device program
# ---------------------------------------------------------------------------

def build_program(n_enc=NE, n_dec=ND):
    from contextlib import ExitStack

    nc = bacc.Bacc(None, target_bir_lowering=False, debug=False)

    with tile.TileContext(nc) as tc, ExitStack() as es:
        dram = es.enter_context(tc.tile_pool(name="dram", bufs=1, space="DRAM"))

        def din(name, shape, dtype):
            return dram.tile(shape, dtype, kind="ExternalInput", name=name,
                             uniquify=False)

        x_enc_idx = din("x_enc_idx", [128, B * n_enc // 16], mybir.dt.int16)
        y_dec_idx = din("y_dec_idx", [128, B * n_dec // 16], mybir.dt.int16)
        e_idx = din("e_idx", [128, BPC * NE // 16], mybir.dt.int16)
        gb_idx = din("gb_idx", [128, BPC * ND // 16], mybir.dt.int16)
        gembed_bf = din("gembed_bf", [V, F], BF)
        enc_embed_bf = din("enc_embed_bf", [V, EE], BF)
        dec_embed_bf = din("dec_embed_bf", [V, ED], BF)
        w2t_bf = din("w2t_bf", [V, F], BF)
        w2_d = din("w2_d", [128, 2, V], BF)
        gconv_d = din("gconv_d", [128, KW * 4, 128], BF)
        wih_d = din("wih_d", [128, 4 * H], BF)
        whh_d = din("whh_d", [128, 2, 4 * H], BF)
        wiy_d = din("wiy_d", [128, 4 * H], BF)
        whd_d = din("whd_d", [128, 2, 4 * H], BF)
        tt_d = din("tt_d", [128, 8, 128], BF)
        pout = dram.tile([128, 16], F32, kind="ExternalOutput", name="pout",
                         uniquify=False)

        cpool = es.enter_context(tc.tile_pool(name="const", bufs=1))

        idbf = cpool.tile([128, 128], BF)
        make_identity(nc, idbf[:])
        idf32 = cpool.tile([128, 128], F32)
        make_identity(nc, idf32[:])
        negones = cpool.tile([1, 128], F32)
        nc.gpsimd.memset(negones[:], -1.0)

        def to_sbuf(ap, name):
            t = cpool.tile(list(ap.shape), ap.dtype, name=name)
            nc.sync.dma_start(out=t[:], in_=ap[:])
            return t

        w2_sb = to_sbuf(w2_d, "w2_sb")
        gconv_sb = to_sbuf(gconv_d, "gconv_sb")
        wih_sb = to_sbuf(wih_d, "wih_sb")
        whh_sb = to_sbuf(whh_d, "whh_sb")
        wiy_sb = to_sbuf(wiy_d, "wiy_sb")
        whd_sb = to_sbuf(whd_d, "whd_sb")
        tt_sb = to_sbuf(tt_d, "tt_sb")
        xidx_sb = to_sbuf(x_enc_idx, "xidx_sb")
        yidx_sb = to_sbuf(y_dec_idx, "yidx_sb")
        eidx_sb = to_sbuf(e_idx, "eidx_sb")
        gbidx_sb = to_sbuf(gb_idx, "gbidx_sb")

        gpool = es.enter_context(tc.tile_pool(name="gath", bufs=1))
        GCH = 4096

        def big_gather(out_t, table, idx_sb, n):
            for k0 in range(0, n, GCH):
                c = min(GCH, n - k0)
                nc.gpsimd.dma_gather(
                    out_ap=out_t[:, :, k0:k0 + c], in_ap=table[:],
                    idxs_ap=idx_sb[:, k0 // 16:(k0 + c) // 16],
                    num_idxs=c, num_idxs_reg=c, elem_size=EE,
                    transpose=True, single_packet=False)

        xgath = gpool.tile([128, 1, B * n_enc], BF)
        big_gather(xgath, enc_embed_bf, xidx_sb, B * n_enc)
        ygath = gpool.tile([128, 1, B * n_dec], BF)
        big_gather(ygath, dec_embed_bf, yidx_sb, B * n_dec)

        eT = [gpool.tile([128, 2, NE], BF, name=f"eT{b}") for b in range(BPC)]
        gbT = [gpool.tile([128, 2, ND], BF, name=f"gbT{b}") for b in range(BPC)]

        # persistent activation stores
        spool = es.enter_context(tc.tile_pool(name="stores", bufs=1))
        tcT = [spool.tile([128, 2, NE], BF, name=f"tcT{b}") for b in range(BPC)]
        lnZ = [spool.tile([1, NE], F32, name=f"lnZ{b}") for b in range(BPC)]
        hencTf = spool.tile([128, 2, BPC * NE], BF)
        hencTb = spool.tile([128, 2, BPC * NE], BF)
        hdecT = spool.tile([128, 2, BPC * (ND + 1)], BF)
        pout_sb = spool.tile([128, 16], F32)
        nc.gpsimd.memset(hencTf[:], 0.0)
        nc.gpsimd.memset(hencTb[:], 0.0)
        nc.gpsimd.memset(hdecT[:], 0.0)

        # ------------------------------------------------------------------
        # Phase G
        # ------------------------------------------------------------------
        with tc.tile_pool(name="gwork", bufs=2) as gw, \
             tc.tile_pool(name="gpsum", bufs=2, space="PSUM") as gp, \
             tc.tile_pool(name="zrow", bufs=4, space="PSUM") as zrp:
            for b in range(BPC):
                gsc = gw.tile([128, 2, NE], BF, tag="gathsc")
                nc.gpsimd.dma_gather(
                    out_ap=gsc[:], in_ap=gembed_bf[:],
                    idxs_ap=eidx_sb[:, b * NE // 16:(b + 1) * NE // 16],
                    num_idxs=NE, num_idxs_reg=NE, elem_size=F, transpose=True)
                nc.scalar.activation(eT[b][:], gsc[:], AF.Tanh)
                nc.gpsimd.dma_gather(
                    out_ap=gbT[b][:], in_ap=w2t_bf[:],
                    idxs_ap=gbidx_sb[:, b * ND // 16:(b + 1) * ND // 16],
                    num_idxs=ND, num_idxs_reg=ND, elem_size=F, transpose=True)
            # conv + tanh
            for b in range(BPC):
                for fo in range(2):
                    cp = gp.tile([128, NE], F32, tag="convps")
                    first = True
                    for k in [2, 0, 1, 3, 4]:
                        d = k - 2
                        lo_out, lo_in = max(0, -d), max(0, d)
                        L = NE - abs(d)
                        for fi in range(2):
                            nc.tensor.matmul(
                                cp[:, lo_out:lo_out + L],
                                gconv_sb[:, (k * 2 + fi) * 2 + fo, :],
                                eT[b][:, fi, lo_in:lo_in + L],
                                start=first, stop=(k == 4 and fi == 1),
                                skip_group_check=True)
                            first = False
                    nc.scalar.activation(tcT[b][:, fo, :], cp[:], AF.Tanh)
            # logits (t-major) -> exp -> Z
            zrows = []
            for b in range(BPC):
                zrow = zrp.tile([1, NE], F32, tag="zrow", name=f"zr{b}")
                for ic in range(4):
                    zp = gw.tile([128, 4], F32, tag="zp")
                    for vc in range(4):
                        lp = gp.tile([128, 500], F32, tag="logps")
                        for f in range(2):
                            nc.tensor.matmul(
                                lp[:], tcT[b][:, f, ic * 128:(ic + 1) * 128],
                                w2_sb[:, f, vc * 500:(vc + 1) * 500],
                                start=(f == 0), stop=(f == 1))
                        sc = gw.tile([128, 500], BF, tag="expsc")
                        nc.scalar.activation(sc[:], lp[:], AF.Exp,
                                             accum_out=zp[:, vc:vc + 1])
                    zc = gw.tile([128, 1], F32, tag="zc")
                    nc.vector.tensor_reduce(zc[:], zp[:],
                                            axis=mybir.AxisListType.X,
                                            op=mybir.AluOpType.add)
                    nc.tensor.transpose(zrow[:, ic * 128:(ic + 1) * 128],
                                        zc[:], idf32[:])
                zrows.append(zrow)
            for b in range(BPC):
                nc.scalar.activation(lnZ[b][:], zrows[b][:], AF.Ln)

        # ------------------------------------------------------------------
        # LSTM phases
        # ------------------------------------------------------------------
        lstm_es = ExitStack()
        lst_sb = lstm_es.enter_context(tc.tile_pool(name="lstm_sb", bufs=2))
        lst_ps = lstm_es.enter_context(tc.tile_pool(name="lstm_ps", bufs=3,
                                                    space="PSUM"))
        lst_tp = lstm_es.enter_context(tc.tile_pool(name="lstm_tp", bufs=2,
                                                    space="PSUM"))

        class Chain:
            def __init__(self, name, Bc, xg, wih, whh, store, col_of,
                         xcol, n_steps):
                self.name, self.B = name, Bc
                self.xg, self.wih, self.whh = xg, wih, whh
                self.store, self.col_of = store, col_of
                self.xcol, self.n_steps = xcol, n_steps
                self.hT = None
                self.W = None  # (Bc, 1280) bf16 = [si sf g~ c so]
                self.z_next = None

            def init_zero(self):
                hT = lst_sb.tile([128, 2, self.B], BF, tag=self.name + "hT")
                W = lst_sb.tile([self.B, 1280], BF, tag=self.name + "W",
                                bufs=1, name="W")
                nc.gpsimd.memset(hT[:], 0.0)
                nc.gpsimd.memset(W[:], 0.0)
                self.hT, self.W = hT, W

            def prime(self, t):
                """Issue the x-part matmuls for step t into a fresh z tile."""
                Bc = self.B
                z = lst_ps.tile([B, 1024], F32, tag="xz", bufs=3,
                                name="z")[0:Bc, :]
                xc = self.xg[:, 0, self.xcol(t):self.xcol(t) + Bc]
                nc.tensor.matmul(z[:, 0:512], xc, self.wih[:, 0:512],
                                 start=True, stop=False,
                                 skip_group_check=True)
                nc.tensor.matmul(z[:, 512:1024], xc, self.wih[:, 512:1024],
                                 start=True, stop=False,
                                 skip_group_check=True)
                self.z_next = z

            def s1(self, t):
                Bc = self.B
                z = self.z_next
                # finish z[0:512] first so sigmoid_if can start early
                for nh in range(2):
                    for hc in range(2):
                        nc.tensor.matmul(z[:, nh * 512:nh * 512 + 512],
                                         self.hT[:, hc, :],
                                         self.whh[:, hc,
                                                  nh * 512:nh * 512 + 512],
                                         start=False, stop=(hc == 1),
                                         skip_group_check=True)
                # W = [si sf | g~ | c | so]
                W = self.W
                nc.scalar.activation(W[:, 0:512], z[:, 0:512], AF.Sigmoid)
                nc.scalar.activation(W[:, 512:768], z[:, 512:768], AF.Tanh)

            def s2(self, t):
                Bc, nm = self.B, self.name
                z, W = self.z_next, self.W
                # m1 = sf*c (only needs sigmoid_if); m0 = si*g~; c' = m0+m1
                m = lst_sb.tile([B, 512], BF, tag="xt2", bufs=3,
                                name="m")[0:Bc, :]
                nc.vector.tensor_mul(m[:, 256:512], W[:, 256:512],
                                     W[:, 768:1024])
                nc.vector.tensor_mul(m[:, 0:256], W[:, 0:256],
                                     W[:, 512:768])
                nc.vector.tensor_add(W[:, 768:1024], m[:, 0:256],
                                     m[:, 256:512])
                # so (off critical path)
                nc.scalar.activation(W[:, 1024:1280], z[:, 768:1024],
                                     AF.Sigmoid)
                # pre-issue next step's x matmuls so they don't queue
                # behind the transposes in PE program order
                if t + 1 < self.n_steps:
                    self.prime(t + 1)
                # transpose c' and so; tanh + h-mul done H-major
                psT = lst_tp.tile([128, 4, B], BF, tag="xpsT", bufs=2,
                                  name="psT")[:, :, 0:Bc]
                nc.tensor.transpose(psT[:, 0, :], W[:, 768:896],
                                    idbf[0:Bc, 0:Bc])
                nc.tensor.transpose(psT[:, 1, :], W[:, 896:1024],
                                    idbf[0:Bc, 0:Bc])
                nc.tensor.transpose(psT[:, 2, :], W[:, 1024:1152],
                                    idbf[0:Bc, 0:Bc])
                nc.tensor.transpose(psT[:, 3, :], W[:, 1152:1280],
                                    idbf[0:Bc, 0:Bc])
                thT = lst_sb.tile([128, 2, B], BF, tag="xthT", bufs=3,
                                  name="thT")[:, :, 0:Bc]
                nc.scalar.activation(thT[:], psT[:, 0:2, :], AF.Tanh)
                hT = lst_sb.tile([128, 2, Bc], BF, tag=nm + "hT")
                nc.vector.tensor_mul(hT[:], psT[:, 2:4, :], thT[:])
                if self.store is not None:
                    col = self.col_of(t)
                    stride = self.store.shape[2] // BPC
                    nc.vector.tensor_copy(
                        self.store[:, :, col::stride][:, :, 0:BPC],
                        hT[:, :, 0:BPC])
                self.hT = hT

        fwd = Chain("f", B, xgath, wih_sb, whh_sb, hencTf, lambda t: t,
                    lambda t: t * B, n_enc)
        bwd = Chain("b", B, xgath, wih_sb, whh_sb, hencTb,
                    lambda t: n_enc - 1 - t,
                    lambda t: (n_enc - 1 - t) * B, n_enc)
        fwd.init_zero()
        bwd.init_zero()
        fwd.prime(0)
        bwd.prime(0)

        bt = 0
        for t in range(n_enc):
            fwd.s1(t)
            do_b = (t % 2 == 1 and bt < n_enc // 2)
            if do_b:
                bwd.s1(bt)
            fwd.s2(t)
            if do_b:
                bwd.s2(bt)
                bt += 1

        hTf, Wf = fwd.hT, fwd.W
        nc.vector.tensor_copy(hdecT[:, :, 0::ND + 1][:, :, 0:BPC],
                              hTf[:, :, 0:BPC])

        dec = Chain("d", B, ygath, wiy_sb, whd_sb, hdecT, lambda t: t + 1,
                    lambda t: t * B, n_dec)
        dec.hT, dec.W = hTf, Wf
        dec.prime(0)

        for t in range(n_dec):
            dec.s1(t)
            do_b = (t % 2 == 1 and bt < n_enc)
            if do_b:
                bwd.s1(bt)
            dec.s2(t)
            if do_b:
                bwd.s2(bt)
                bt += 1
        while bt < n_enc:
            bwd.s1(bt)
            bwd.s2(bt)
            bt += 1

        lstm_es.close()

        # ------------------------------------------------------------------
        # Final phase
        # ------------------------------------------------------------------
        with tc.tile_pool(name="fin_sb", bufs=2) as fsb, \
             tc.tile_pool(name="fin_keep", bufs=1) as fkeep, \
             tc.tile_pool(name="fin_ps", bufs=2, space="PSUM") as fps:
            sda = [fkeep.tile([128, 8], F32, name=f"sda{b}")
                   for b in range(BPC)]
            for b in range(BPC):
                thT = fsb.tile([128, 2, NE], BF, tag="thT")
                for hc in range(2):
                    tp = fps.tile([128, NE], F32, tag="thps")
                    for ec in range(4):
                        src = hencTf if ec < 2 else hencTb
                        nc.tensor.matmul(
                            tp[:], tt_sb[:, ec * 2 + hc, :],
                            src[:, ec % 2, b * NE:(b + 1) * NE],
                            start=(ec == 0), stop=(ec == 3))
                    nc.scalar.activation(thT[:, hc, :], tp[:], AF.Copy)
                for jc in range(4):
                    fp = fps.tile([128, NE], F32, tag="fps")
                    for hc in range(2):
                        nc.tensor.matmul(
                            fp[:],
                            hdecT[:, hc, :][:, b * (ND + 1) + jc * 128:
                                            b * (ND + 1) + jc * 128 + 128],
                            thT[:, hc, :], start=(hc == 0), stop=False,
                            skip_group_check=True)
                    sc1 = fsb.tile([128, NE], BF, tag="fexp")
                    nc.scalar.activation(
                        sc1[:], fp[:], AF.Exp,
                        accum_out=sda[b][:, 2 * jc:2 * jc + 1])
                    for f in range(2):
                        nc.tensor.matmul(
                            fp[:], gbT[b][:, f, jc * 128:jc * 128 + 128],
                            tcT[b][:, f, :], start=False, stop=False,
                            skip_group_check=True)
                    nc.tensor.matmul(fp[:], negones[:, 0:128], lnZ[b][:],
                                     start=False, stop=True,
                                     skip_group_check=True)
                    sc2 = fsb.tile([128, NE], BF, tag="fexp")
                    nc.scalar.activation(
                        sc2[:], fp[:], AF.Exp,
                        accum_out=sda[b][:, 2 * jc + 1:2 * jc + 2])
            for b in range(BPC):
                lns = fsb.tile([128, 8], F32, tag="lns")
                nc.scalar.activation(lns[:], sda[b][:], AF.Ln)
                for jc in range(4):
                    nc.vector.tensor_sub(
                        pout_sb[:, b * 4 + jc:b * 4 + jc + 1],
                        lns[:, 2 * jc + 1:2 * jc + 2],
                        lns[:, 2 * jc:2 * jc + 1])
            nc.sync.dma_start(out=pout[:], in_=pout_sb[:])

    nc.compile()
    return nc


# ---------------------------------------------------------------------------
# host side
# ---------------------------------------------------------------------------

_CACHE = {}


def _get_program(n_enc, n_dec):
    key = (n_enc, n_dec)
    if key not in _CACHE:
        _CACHE[key] = build_program(n_enc, n_dec)
    return _CACHE[key]


def _host_prep(inputs, n_enc=NE, n_dec=ND):
    xs = np.asarray(inputs["xs_idx"]).astype(np.int64)
    ys = np.asarray(inputs["ys_idx"]).astype(np.int64)
    gembed_W = np.asarray(inputs["gembed_W"], np.float32)
    gconv_W = np.asarray(inputs["gconv_W"], np.float32)
    gdecode_W = np.asarray(inputs["gdecode_W"], np.float32)
    enc_embed = np.asarray(inputs["enc_embed"], np.float32)
    dec_embed = np.asarray(inputs["dec_embed"], np.float32)
    T = np.asarray(inputs["T"], np.float32)

    for nm in ("enc_b", "dec_b"):
        assert not np.any(np.asarray(inputs[nm])), f"{nm} nonzero unsupported"

    def lstm_w(wih, whh):
        wih = np.asarray(wih, np.float32)
        whh = np.asarray(whh, np.float32)
        wih_t = _bf(wih.T)
        whh_t = _bf(whh.T.reshape(2, 128, 4 * H).transpose(1, 0, 2))
        return wih_t, whh_t

    wih_d, whh_d = lstm_w(inputs["enc_Wih"], inputs["enc_Whh"])
    wiy_d, whd_d = lstm_w(inputs["dec_Wih"], inputs["dec_Whh"])

    w2_d = _bf(gdecode_W.reshape(2, 128, V).transpose(1, 0, 2))
    g = gconv_W.reshape(KW, 2, 128, 2, 128)
    gconv_d = _bf(np.ascontiguousarray(
        g.transpose(2, 0, 1, 3, 4).reshape(128, KW * 4, 128)))
    tt = T.T.reshape(4, 128, 2, 128)  # [ec, p, hc, c]
    tt_d = _bf(np.ascontiguousarray(
        tt.transpose(1, 0, 2, 3).reshape(128, 8, 128)))

    base = dict(
        gembed_bf=_bf(gembed_W), enc_embed_bf=_bf(enc_embed),
        dec_embed_bf=_bf(dec_embed), w2t_bf=_bf(gdecode_W.T),
        w2_d=w2_d, gconv_d=gconv_d, wih_d=wih_d, whh_d=whh_d,
        wiy_d=wiy_d, whd_d=whd_d, tt_d=tt_d,
    )

    in_maps = []
    for m in range(NCORES):
        order = np.concatenate(
            [np.arange(4 * m, 4 * m + 4),
             np.delete(np.arange(B), np.s_[4 * m:4 * m + 4])])
        xs_p, ys_p = xs[order], ys[order]
        xm = np.where(xs_p < PG, 0, xs_p)
        ym = np.where(ys_p < PG, 0, ys_p)
        im = dict(base)
        im["x_enc_idx"] = _wrap16(xm[:, :n_enc].T)   # (t,b) order
        im["y_dec_idx"] = _wrap16(ym[:, :n_dec].T)
        im["e_idx"] = _wrap16(xs_p[:BPC])            # (b,t) order
        im["gb_idx"] = _wrap16(ys_p[:BPC])
        in_maps.append(im)
    return in_maps


def kernel(**inputs):
    trace = bool(int(os.environ.get("KERNEL_TRACE", "0")))
    n_enc = int(os.environ.get("KERNEL_NENC", NE))
    n_dec = int(os.environ.get("KERNEL_NDEC", ND))
    nc = _get_program(n_enc, n_dec)
    in_maps = _host_prep(inputs, n_enc, n_dec)
    res = run_bass_kernel_spmd(nc, in_maps, list(range(NCORES)), trace=trace)
    total = np.float64(0.0)
    for r in res.results:
        total += np.asarray(r["pout"], np.float64).sum()
    kernel.last_results = res
    return np.float32(-total)



# revision 9
# speedup vs baseline: 1.5947x; 1.5947x over previous
"""Trainium2 Bass kernel for nn_EquivariantHardAlignmentModel.

8 NeuronCores, SPMD (identical program, per-core data):
  - LSTM recurrences run H-major / weight-stationary: each step streams the
    hidden state (and the gathered x embedding) through 24 stationary
    128x128 weight tiles, so gates land on full 128 partitions, no PE
    transposes are needed, and per-step PE cost is the LDWEIGHTS floor.
    enc-fwd and enc-bwd share every weight load (64 moving columns/step);
    the decoder runs the same way in a second phase.
  - The G-stack (embed/conv/logits/Z), ys gathers, bilinear alignment and
    loss tail are data-parallel: each core does 4 of 32 batch rows.  Inputs
    are batch-permuted per core so its rows are always rows 0..3 -> one
    shared program.
  - p[b,j] = log(sum_i exp(lys+eij-lnZ)) - log(sum_i exp(eij)) via
    PSUM-accumulated matmuls + ACT Exp(accum_out).  Host sums & negates.
Phase order: LSTM-A (fwd+bwd) -> G -> LSTM-B (dec) -> final, so the PE
never waits on the gpsimd gathers that feed G.
"""

import os
import sys

sys.path.insert(0, "/opt/trn_rl_repo")

import numpy as np
import ml_dtypes

import concourse.bass as bass
import concourse.mybir as mybir
import concourse.tile as tile
from concourse import bacc
from concourse.bass_utils import run_bass_kernel_spmd
from concourse.masks import make_identity

BF = mybir.dt.bfloat16
F32 = mybir.dt.float32
AF = mybir.ActivationFunctionType

B, NE, ND = 32, 512, 512
V = 2000
H, F, KW, PG = 256, 256, 5, 4
EE, ED = 128, 128
NCORES, BPC = 8, 4
XCH = 4096  # columns per x-gather chunk tile (128 steps * 32 batch)

# gate -> (n-tile pair) in PyTorch i,f,g,o row order
GATE_NT = (("g", (4, 5)), ("f", (2, 3)), ("i", (0, 1)), ("o", (6, 7)))


def _bf(x):
    return np.ascontiguousarray(x.astype(ml_dtypes.bfloat16))


def _wrap16(flat):
    """index list -> (128, n/16) int16, dma_gather wrapped + 8x replicated."""
    flat = np.asarray(flat).reshape(-1)
    assert flat.size % 16 == 0
    w = flat.reshape(-1, 16).T.astype(np.int16)  # (16, n/16)
    return np.ascontiguousarray(np.tile(w, (8, 1)))


# ---------------------------------------------------------------------------
# device program
# ---------------------------------------------------------------------------

def build_program(n_enc=NE, n_dec=ND):
    from contextlib import ExitStack

    nc = bacc.Bacc(None, target_bir_lowering=False, debug=False)
    n_xc = n_enc * B // XCH  # x chunk tiles per sequence
    n_yc = n_dec * B // XCH

    with tile.TileContext(nc) as tc, ExitStack() as es:
        dram = es.enter_context(tc.tile_pool(name="dram", bufs=1, space="DRAM"))

        def din(name, shape, dtype):
            return dram.tile(shape, dtype, kind="ExternalInput", name=name,
                             uniquify=False)

        x_enc_idx = din("x_enc_idx", [128, B * n_enc // 16], mybir.dt.int16)
        y_dec_idx = din("y_dec_idx", [128, B * n_dec // 16], mybir.dt.int16)
        e_idx = din("e_idx", [128, BPC * NE // 16], mybir.dt.int16)
        gb_idx = din("gb_idx", [128, BPC * ND // 16], mybir.dt.int16)
        gembed_bf = din("gembed_bf", [V, F], BF)
        enc_embed_bf = din("enc_embed_bf", [V, EE], BF)
        dec_embed_bf = din("dec_embed_bf", [V, ED], BF)
        w2t_bf = din("w2t_bf", [V, F], BF)
        w2_d = din("w2_d", [128, 2, V], BF)
        gconv_d = din("gconv_d", [128, KW * 4, 128], BF)
        # H-major weight tiles: wih [128E, nt, 128n]; whh [128k, nt*2+kc, 128n]
        wih_e_d = din("wih_e_d", [128, 8, 128], BF)
        whh_e_d = din("whh_e_d", [128, 16, 128], BF)
        wih_d_d = din("wih_d_d", [128, 8, 128], BF)
        whh_d_d = din("whh_d_d", [128, 16, 128], BF)
        tt_d = din("tt_d", [128, 8, 128], BF)
        pout = dram.tile([128, 16], F32, kind="ExternalOutput", name="pout",
                         uniquify=False)

        cpool = es.enter_context(tc.tile_pool(name="const", bufs=1))

        idf32 = cpool.tile([128, 128], F32)
        make_identity(nc, idf32[:])
        negones = cpool.tile([1, 128], F32)
        nc.gpsimd.memset(negones[:], -1.0)

        def to_sbuf(ap, name):
            t = cpool.tile(list(ap.shape), ap.dtype, name=name)
            nc.sync.dma_start(out=t[:], in_=ap[:])
            return t

        w2_sb = to_sbuf(w2_d, "w2_sb")
        gconv_sb = to_sbuf(gconv_d, "gconv_sb")
        wih_e = to_sbuf(wih_e_d, "wih_e")
        whh_e = to_sbuf(whh_e_d, "whh_e")
        wih_dd = to_sbuf(wih_d_d, "wih_dd")
        whh_dd = to_sbuf(whh_d_d, "whh_dd")
        tt_sb = to_sbuf(tt_d, "tt_sb")
        xidx_sb = to_sbuf(x_enc_idx, "xidx_sb")
        yidx_sb = to_sbuf(y_dec_idx, "yidx_sb")
        eidx_sb = to_sbuf(e_idx, "eidx_sb")
        gbidx_sb = to_sbuf(gb_idx, "gbidx_sb")

        # zero LSTM init state: must hit the gpsimd queue BEFORE the big
        # gathers, or phase A's first step waits ~500us behind them
        hc0 = cpool.tile([128, 2, 64], BF, name="hc0")
        cc0 = cpool.tile([128, 2, 64], BF, name="cc0")
        nc.gpsimd.memset(hc0[:], 0.0)
        nc.gpsimd.memset(cc0[:], 0.0)

        gpool = es.enter_context(tc.tile_pool(name="gath", bufs=1))

        def chunk_gather(table, idx_sb, nchunks, name):
            tiles = []
            for k in range(nchunks):
                t = gpool.tile([128, 1, XCH], BF, name=f"{name}{k}")
                tiles.append(t)
            return tiles

        xgc = chunk_gather(enc_embed_bf, xidx_sb, n_xc, "xg")
        ygc = chunk_gather(dec_embed_bf, yidx_sb, n_yc, "yg")

        def issue_gather(tiles, table, idx_sb, order):
            for k in order:
                nc.gpsimd.dma_gather(
                    out_ap=tiles[k][:, :, :], in_ap=table[:],
                    idxs_ap=idx_sb[:, k * XCH // 16:(k + 1) * XCH // 16],
                    num_idxs=XCH, num_idxs_reg=XCH, elem_size=EE,
                    transpose=True, single_packet=False)

        # fwd needs chunk 0 first, bwd needs the last chunk first
        xorder = list(range(n_xc))
        if n_xc > 1:
            xorder = [xorder[0], xorder[-1]] + xorder[1:-1]
        issue_gather(xgc, enc_embed_bf, xidx_sb, xorder)

        eT = [gpool.tile([128, 2, NE], BF, name=f"eT{b}") for b in range(BPC)]
        gbT = [gpool.tile([128, 2, ND], BF, name=f"gbT{b}") for b in range(BPC)]
        for b in range(BPC):
            nc.gpsimd.dma_gather(
                out_ap=eT[b][:], in_ap=gembed_bf[:],
                idxs_ap=eidx_sb[:, b * NE // 16:(b + 1) * NE // 16],
                num_idxs=NE, num_idxs_reg=NE, elem_size=F, transpose=True)
            nc.gpsimd.dma_gather(
                out_ap=gbT[b][:], in_ap=w2t_bf[:],
                idxs_ap=gbidx_sb[:, b * ND // 16:(b + 1) * ND // 16],
                num_idxs=ND, num_idxs_reg=ND, elem_size=F, transpose=True)

        issue_gather(ygc, dec_embed_bf, yidx_sb, list(range(n_yc)))

        # persistent activation stores
        spool = es.enter_context(tc.tile_pool(name="stores", bufs=1))
        tcT = [spool.tile([128, 2, NE], BF, name=f"tcT{b}") for b in range(BPC)]
        lnZ = [spool.tile([1, NE], F32, name=f"lnZ{b}") for b in range(BPC)]
        hencTf = spool.tile([128, 2, BPC * NE], BF)
        hencTb = spool.tile([128, 2, BPC * NE], BF)
        hdecT = spool.tile([128, 2, BPC * (ND + 1)], BF)
        pout_sb = spool.tile([128, 16], F32)

        # ------------------------------------------------------------------
        # LSTM step: H-major, weight-stationary
        # ------------------------------------------------------------------
        lstm_sb = es.enter_context(tc.tile_pool(name="lstm_sb", bufs=2))

        def lstm_step(psp, W, h_prev, c_prev, whh_sb, wih_sb, x_slices):
            """One fused LSTM step; returns (h_new, c_new) [128, 2, 64] bf16.

            W: moving-column count (64 = fwd+bwd fused, 32 = dec).
            x_slices: [(chunk_tile, col0, out_off, width), ...]
            """
            ps = {}
            sb = {}

            def gate_mms(gname, nts):
                p = psp.tile([128, 2, 256], F32, tag="ps" + gname)
                ps[gname] = p
                for j, nt in enumerate(nts):
                    nc.tensor.matmul(p[:, j, 0:W], whh_sb[:, nt * 2, :],
                                     h_prev[:, 0, 0:W], start=True,
                                     stop=False, skip_group_check=True)
                    nc.tensor.matmul(p[:, j, 0:W], whh_sb[:, nt * 2 + 1, :],
                                     h_prev[:, 1, 0:W], start=False,
                                     stop=False, skip_group_check=True)
                    for si_, (xt, c0, off, w) in enumerate(x_slices):
                        nc.tensor.matmul(p[:, j, off:off + w],
                                         wih_sb[:, nt, :],
                                         xt[:, 0, c0:c0 + w],
                                         start=False,
                                         stop=(si_ == len(x_slices) - 1),
                                         skip_group_check=True)

            def act(gname, func):
                t = lstm_sb.tile([128, 2, 64], BF, tag="a" + gname)
                sb[gname] = t
                nc.scalar.activation(t[:, :, 0:W], ps[gname][:, :, 0:W], func)

            gate_mms("g", GATE_NT[0][1])
            act("g", AF.Tanh)
            gate_mms("f", GATE_NT[1][1])
            act("f", AF.Sigmoid)
            m1 = lstm_sb.tile([128, 2, 64], BF, tag="m1")
            nc.vector.tensor_mul(m1[:, :, 0:W], sb["f"][:, :, 0:W],
                                 c_prev[:, :, 0:W])
            gate_mms("i", GATE_NT[2][1])
            act("i", AF.Sigmoid)
            m0 = lstm_sb.tile([128, 2, 64], BF, tag="m0")
            nc.vector.tensor_mul(m0[:, :, 0:W], sb["i"][:, :, 0:W],
                                 sb["g"][:, :, 0:W])
            gate_mms("o", GATE_NT[3][1])
            c_new = lstm_sb.tile([128, 2, 64], BF, tag="c")
            nc.vector.tensor_add(c_new[:, :, 0:W], m0[:, :, 0:W],
                                 m1[:, :, 0:W])
            act("o", AF.Sigmoid)
            tc_ = lstm_sb.tile([128, 2, 64], BF, tag="tc")
            nc.scalar.activation(tc_[:, :, 0:W], c_new[:, :, 0:W], AF.Tanh)
            h_new = lstm_sb.tile([128, 2, 64], BF, tag="h", bufs=3)
            nc.vector.tensor_mul(h_new[:, :, 0:W], sb["o"][:, :, 0:W],
                                 tc_[:, :, 0:W])
            return h_new, c_new

        # ------------------------------------------------------------------
        # Phase A: enc fwd + enc bwd, fused 64 moving columns
        # ------------------------------------------------------------------
        with tc.tile_pool(name="psA", bufs=2, space="PSUM") as psA:
            h, c = hc0, cc0
            for t in range(n_enc):
                tb = n_enc - 1 - t
                xs = [(xgc[(t * B) // XCH], (t * B) % XCH, 0, 32),
                      (xgc[(tb * B) // XCH], (tb * B) % XCH, 32, 32)]
                h, c = lstm_step(psA, 64, h, c, whh_e, wih_e, xs)
                nc.vector.tensor_copy(hencTf[:, :, t::NE][:, :, 0:BPC],
                                      h[:, :, 0:BPC])
                nc.vector.tensor_copy(hencTb[:, :, tb::NE][:, :, 0:BPC],
                                      h[:, :, 32:32 + BPC])
            h_fin, c_fin = h, c
            nc.vector.tensor_copy(hdecT[:, :, 0::ND + 1][:, :, 0:BPC],
                                  h_fin[:, :, 0:BPC])

        # ------------------------------------------------------------------
        # Phase G (between the LSTM phases; its gathers ran during A)
        # ------------------------------------------------------------------
        with tc.tile_pool(name="gwork", bufs=2) as gw, \
             tc.tile_pool(name="gpsum", bufs=2, space="PSUM") as gp, \
             tc.tile_pool(name="zrow", bufs=4, space="PSUM") as zrp:
            # e = tanh(gembed[xs]) — emitted here (not at gather time) so it
            # does not block phase A's ACT stream behind the gpsimd gathers
            etan = [gpool.tile([128, 2, NE], BF, name=f"etan{b}")
                    for b in range(BPC)]
            for b in range(BPC):
                nc.scalar.activation(etan[b][:], eT[b][:], AF.Tanh)
            # conv + tanh
            for b in range(BPC):
                for fo in range(2):
                    cp = gp.tile([128, NE], F32, tag="convps")
                    first = True
                    for k in [2, 0, 1, 3, 4]:
                        d = k - 2
                        lo_out, lo_in = max(0, -d), max(0, d)
                        L = NE - abs(d)
                        for fi in range(2):
                            nc.tensor.matmul(
                                cp[:, lo_out:lo_out + L],
                                gconv_sb[:, (k * 2 + fi) * 2 + fo, :],
                                etan[b][:, fi, lo_in:lo_in + L],
                                start=first, stop=(k == 4 and fi == 1),
                                skip_group_check=True)
                            first = False
                    nc.scalar.activation(tcT[b][:, fo, :], cp[:], AF.Tanh)
            # logits (t-major) -> exp -> Z
            zrows = []
            for b in range(BPC):
                zrow = zrp.tile([1, NE], F32, tag="zrow", name=f"zr{b}")
                for ic in range(4):
                    zp = gw.tile([128, 4], F32, tag="zp")
                    for vc in range(4):
                        lp = gp.tile([128, 500], F32, tag="logps")
                        for f in range(2):
                            nc.tensor.matmul(
                                lp[:], tcT[b][:, f, ic * 128:(ic + 1) * 128],
                                w2_sb[:, f, vc * 500:(vc + 1) * 500],
                                start=(f == 0), stop=(f == 1))
                        sc = gw.tile([128, 500], BF, tag="expsc")
                        nc.scalar.activation(sc[:], lp[:], AF.Exp,
                                             accum_out=zp[:, vc:vc + 1])
                    zc = gw.tile([128, 1], F32, tag="zc")
                    nc.vector.tensor_reduce(zc[:], zp[:],
                                            axis=mybir.AxisListType.X,
                                            op=mybir.AluOpType.add)
                    nc.tensor.transpose(zrow[:, ic * 128:(ic + 1) * 128],
                                        zc[:], idf32[:])
                zrows.append(zrow)
            for b in range(BPC):
                nc.scalar.activation(lnZ[b][:], zrows[b][:], AF.Ln)

        # ------------------------------------------------------------------
        # Phase B: decoder
        # ------------------------------------------------------------------
        with tc.tile_pool(name="psB", bufs=2, space="PSUM") as psB:
            h, c = h_fin, c_fin
            for t in range(n_dec):
                xs = [(ygc[(t * B) // XCH], (t * B) % XCH, 0, 32)]
                h, c = lstm_step(psB, 32, h, c, whh_dd, wih_dd, xs)
                nc.vector.tensor_copy(hdecT[:, :, t + 1::ND + 1][:, :, 0:BPC],
                                      h[:, :, 0:BPC])

        # ------------------------------------------------------------------
        # Final phase
        # ------------------------------------------------------------------
        with tc.tile_pool(name="fin_sb", bufs=2) as fsb, \
             tc.tile_pool(name="fin_keep", bufs=1) as fkeep, \
             tc.tile_pool(name="fin_ps", bufs=2, space="PSUM") as fps:
            sda = [fkeep.tile([128, 8], F32, name=f"sda{b}")
                   for b in range(BPC)]
            for b in range(BPC):
                thT = fsb.tile([128, 2, NE], BF, tag="thT")
                for hc in range(2):
                    tp = fps.tile([128, NE], F32, tag="thps")
                    for ec in range(4):
                        src = hencTf if ec < 2 else hencTb
                        nc.tensor.matmul(
                            tp[:], tt_sb[:, ec * 2 + hc, :],
                            src[:, ec % 2, b * NE:(b + 1) * NE],
                            start=(ec == 0), stop=(ec == 3))
                    nc.scalar.activation(thT[:, hc, :], tp[:], AF.Copy)
                for jc in range(4):
                    fp = fps.tile([128, NE], F32, tag="fps")
                    for hc in range(2):
                        nc.tensor.matmul(
                            fp[:],
                            hdecT[:, hc, :][:, b * (ND + 1) + jc * 128:
                                            b * (ND + 1) + jc * 128 + 128],
                            thT[:, hc, :], start=(hc == 0), stop=False,
                            skip_group_check=True)
                    sc1 = fsb.tile([128, NE], BF, tag="fexp")
                    nc.scalar.activation(
                        sc1[:], fp[:], AF.Exp,
                        accum_out=sda[b][:, 2 * jc:2 * jc + 1])
                    for f in range(2):
                        nc.tensor.matmul(
                            fp[:], gbT[b][:, f, jc * 128:jc * 128 + 128],
                            tcT[b][:, f, :], start=False, stop=False,
                            skip_group_check=True)
                    nc.tensor.matmul(fp[:], negones[:, 0:128], lnZ[b][:],
                                     start=False, stop=True,
                                     skip_group_check=True)
                    sc2 = fsb.tile([128, NE], BF, tag="fexp")
                    nc.scalar.activation(
                        sc2[:], fp[:], AF.Exp,
                        accum_out=sda[b][:, 2 * jc + 1:2 * jc + 2])
            for b in range(BPC):
                lns = fsb.tile([128, 8], F32, tag="lns")
                nc.scalar.activation(lns[:], sda[b][:], AF.Ln)
                for jc in range(4):
                    nc.vector.tensor_sub(
                        pout_sb[:, b * 4 + jc:b * 4 + jc + 1],
                        lns[:, 2 * jc + 1:2 * jc + 2],
                        lns[:, 2 * jc:2 * jc + 1])
            nc.sync.dma_start(out=pout[:], in_=pout_sb[:])

    nc.compile()
    return nc


# ---------------------------------------------------------------------------
# host side
# ---------------------------------------------------------------------------

_CACHE = {}


def _get_program(n_enc, n_dec):
    key = (n_enc, n_dec)
    if key not in _CACHE:
        _CACHE[key] = build_program(n_enc, n_dec)
    return _CACHE[key]


def _host_prep(inputs, n_enc=NE, n_dec=ND):
    xs = np.asarray(inputs["xs_idx"]).astype(np.int64)
    ys = np.asarray(inputs["ys_idx"]).astype(np.int64)
    gembed_W = np.asarray(inputs["gembed_W"], np.float32)
    gconv_W = np.asarray(inputs["gconv_W"], np.float32)
    gdecode_W = np.asarray(inputs["gdecode_W"], np.float32)
    enc_embed = np.asarray(inputs["enc_embed"], np.float32)
    dec_embed = np.asarray(inputs["dec_embed"], np.float32)
    T = np.asarray(inputs["T"], np.float32)

    for nm in ("enc_b", "dec_b"):
        assert not np.any(np.asarray(inputs[nm])), f"{nm} nonzero unsupported"

    def lstm_w(wih, whh):
        wih = np.asarray(wih, np.float32)  # (4H, E)
        whh = np.asarray(whh, np.float32)  # (4H, H)
        wih_t = _bf(wih.T.reshape(128, 8, 128))
        whh_t = _bf(whh.T.reshape(2, 128, 8, 128)
                    .transpose(1, 2, 0, 3).reshape(128, 16, 128))
        return wih_t, whh_t

    wih_e_d, whh_e_d = lstm_w(inputs["enc_Wih"], inputs["enc_Whh"])
    wih_d_d, whh_d_d = lstm_w(inputs["dec_Wih"], inputs["dec_Whh"])

    w2_d = _bf(gdecode_W.reshape(2, 128, V).transpose(1, 0, 2))
    g = gconv_W.reshape(KW, 2, 128, 2, 128)
    gconv_d = _bf(np.ascontiguousarray(
        g.transpose(2, 0, 1, 3, 4).reshape(128, KW * 4, 128)))
    tt = T.T.reshape(4, 128, 2, 128)  # [ec, p, hc, c]
    tt_d = _bf(np.ascontiguousarray(
        tt.transpose(1, 0, 2, 3).reshape(128, 8, 128)))

    base = dict(
        gembed_bf=_bf(gembed_W), enc_embed_bf=_bf(enc_embed),
        dec_embed_bf=_bf(dec_embed), w2t_bf=_bf(gdecode_W.T),
        w2_d=w2_d, gconv_d=gconv_d,
        wih_e_d=wih_e_d, whh_e_d=whh_e_d,
        wih_d_d=wih_d_d, whh_d_d=whh_d_d, tt_d=tt_d,
    )

    in_maps = []
    for m in range(NCORES):
        order = np.concatenate(
            [np.arange(4 * m, 4 * m + 4),
             np.delete(np.arange(B), np.s_[4 * m:4 * m + 4])])
        xs_p, ys_p = xs[order], ys[order]
        xm = np.where(xs_p < PG, 0, xs_p)
        ym = np.where(ys_p < PG, 0, ys_p)
        im = dict(base)
        im["x_enc_idx"] = _wrap16(xm[:, :n_enc].T)   # (t,b) order
        im["y_dec_idx"] = _wrap16(ym[:, :n_dec].T)
        im["e_idx"] = _wrap16(xs_p[:BPC])            # (b,t) order
        im["gb_idx"] = _wrap16(ys_p[:BPC])
        in_maps.append(im)
    return in_maps


def kernel(**inputs):
    trace = bool(int(os.environ.get("KERNEL_TRACE", "0")))
    n_enc = int(os.environ.get("KERNEL_NENC", NE))
    n_dec = int(os.environ.get("KERNEL_NDEC", ND))
    nc = _get_program(n_enc, n_dec)
    in_maps = _host_prep(inputs, n_enc, n_dec)
    res = run_bass_kernel_spmd(nc, in_maps, list(range(NCORES)), trace=trace)
    total = np.float64(0.0)
    for r in res.results:
        total += np.asarray(r["pout"], np.float64).sum()
    kernel.last_results = res
    return np.float32(-total)


# revision 15
# speedup vs baseline: 1.7428x; 1.0929x over previous
"""Trainium2 Bass kernel for nn_EquivariantHardAlignmentModel.

8 NeuronCores, SPMD (identical program, per-core data):
  - LSTM recurrences run H-major / weight-stationary: each step streams the
    hidden state (and the gathered x embedding) through 24 stationary
    128x128 weight tiles, so gates land on full 128 partitions, no PE
    transposes are needed, and per-step PE cost is the LDWEIGHTS floor.
    enc-fwd and enc-bwd share every weight load (64 moving columns/step);
    the decoder runs the same way in a second phase.
  - The G-stack (embed/conv/logits/Z), ys gathers, bilinear alignment and
    loss tail are data-parallel: each core does 4 of 32 batch rows.  Inputs
    are batch-permuted per core so its rows are always rows 0..3 -> one
    shared program.
  - p[b,j] = log(sum_i exp(lys+eij-lnZ)) - log(sum_i exp(eij)) via
    PSUM-accumulated matmuls + ACT Exp(accum_out).  Host sums & negates.
Phase order: LSTM-A (fwd+bwd) -> G -> LSTM-B (dec) -> final, so the PE
never waits on the gpsimd gathers that feed G.
"""

import os
import sys

sys.path.insert(0, "/opt/trn_rl_repo")

import numpy as np
import ml_dtypes

import concourse.bass as bass
import concourse.mybir as mybir
import concourse.tile as tile
from concourse import bacc
from concourse.bass_utils import run_bass_kernel_spmd
from concourse.masks import make_identity

BF = mybir.dt.bfloat16
F32 = mybir.dt.float32
AF = mybir.ActivationFunctionType

B, NE, ND = 32, 512, 512
V = 2000
H, F, KW, PG = 256, 256, 5, 4
EE, ED = 128, 128
NCORES, BPC = 8, 4
XCH = 4096  # columns per x-gather chunk tile (128 steps * 32 batch)

# gate -> (n-tile pair) in PyTorch i,f,g,o row order
GATE_NT = (("g", (4, 5)), ("f", (2, 3)), ("i", (0, 1)), ("o", (6, 7)))


def _bf(x):
    return np.ascontiguousarray(x.astype(ml_dtypes.bfloat16))


def _wrap16(flat):
    """index list -> (128, n/16) int16, dma_gather wrapped + 8x replicated."""
    flat = np.asarray(flat).reshape(-1)
    assert flat.size % 16 == 0
    w = flat.reshape(-1, 16).T.astype(np.int16)  # (16, n/16)
    return np.ascontiguousarray(np.tile(w, (8, 1)))


# ---------------------------------------------------------------------------
# device program
# ---------------------------------------------------------------------------

def build_program(n_enc=NE, n_dec=ND):
    from contextlib import ExitStack

    nc = bacc.Bacc(None, target_bir_lowering=False, debug=False)
    xch = min(XCH, n_enc * B)  # columns per x chunk tile
    n_xc = n_enc * B // xch  # x chunk tiles per sequence
    n_yc = n_dec * B // xch

    with tile.TileContext(nc) as tc, ExitStack() as es:
        dram = es.enter_context(tc.tile_pool(name="dram", bufs=1, space="DRAM"))

        def din(name, shape, dtype):
            return dram.tile(shape, dtype, kind="ExternalInput", name=name,
                             uniquify=False)

        x_enc_idx = din("x_enc_idx", [128, B * n_enc // 16], mybir.dt.int16)
        y_dec_idx = din("y_dec_idx", [128, B * n_dec // 16], mybir.dt.int16)
        e_idx = din("e_idx", [128, BPC * NE // 16], mybir.dt.int16)
        gb_idx = din("gb_idx", [128, BPC * ND // 16], mybir.dt.int16)
        gembed_bf = din("gembed_bf", [V, F], BF)
        enc_embed_bf = din("enc_embed_bf", [V, EE], BF)
        dec_embed_bf = din("dec_embed_bf", [V, ED], BF)
        w2t_bf = din("w2t_bf", [V, F], BF)
        w2_d = din("w2_d", [128, 2, V], BF)
        gconv_d = din("gconv_d", [128, KW * 4, 128], BF)
        # H-major weight tiles: wih [128E, nt, 128n]; whh [128k, nt*2+kc, 128n]
        wih_e_d = din("wih_e_d", [128, 8, 128], BF)
        whh_e_d = din("whh_e_d", [128, 16, 128], BF)
        wih_d_d = din("wih_d_d", [128, 8, 128], BF)
        whh_d_d = din("whh_d_d", [128, 16, 128], BF)
        tt_d = din("tt_d", [128, 8, 128], BF)
        pout = dram.tile([128, 16], F32, kind="ExternalOutput", name="pout",
                         uniquify=False)

        cpool = es.enter_context(tc.tile_pool(name="const", bufs=1))

        idf32 = cpool.tile([128, 128], F32)
        make_identity(nc, idf32[:])
        negones = cpool.tile([1, 128], F32)
        nc.gpsimd.memset(negones[:], -1.0)

        def to_sbuf(ap, name):
            t = cpool.tile(list(ap.shape), ap.dtype, name=name)
            nc.sync.dma_start(out=t[:], in_=ap[:])
            return t

        w2_sb = to_sbuf(w2_d, "w2_sb")
        gconv_sb = to_sbuf(gconv_d, "gconv_sb")
        wih_e = to_sbuf(wih_e_d, "wih_e")
        whh_e = to_sbuf(whh_e_d, "whh_e")
        wih_dd = to_sbuf(wih_d_d, "wih_dd")
        whh_dd = to_sbuf(whh_d_d, "whh_dd")
        tt_sb = to_sbuf(tt_d, "tt_sb")
        xidx_sb = to_sbuf(x_enc_idx, "xidx_sb")
        yidx_sb = to_sbuf(y_dec_idx, "yidx_sb")
        eidx_sb = to_sbuf(e_idx, "eidx_sb")
        gbidx_sb = to_sbuf(gb_idx, "gbidx_sb")

        # zero LSTM init state: must hit the gpsimd queue BEFORE the big
        # gathers, or phase A's first step waits ~500us behind them
        hc0 = cpool.tile([128, 2, 64], BF, name="hc0")
        cc0 = cpool.tile([128, 2, 64], BF, name="cc0")
        nc.gpsimd.memset(hc0[:], 0.0)
        nc.gpsimd.memset(cc0[:], 0.0)

        gpool = es.enter_context(tc.tile_pool(name="gath", bufs=1))

        def chunk_gather(table, idx_sb, nchunks, name):
            tiles = []
            for k in range(nchunks):
                t = gpool.tile([128, 1, xch], BF, name=f"{name}{k}")
                tiles.append(t)
            return tiles

        xgc = chunk_gather(enc_embed_bf, xidx_sb, n_xc, "xg")
        ygc = chunk_gather(dec_embed_bf, yidx_sb, n_yc, "yg")

        def issue_gather(tiles, table, idx_sb, order):
            for k in order:
                nc.gpsimd.dma_gather(
                    out_ap=tiles[k][:, :, :], in_ap=table[:],
                    idxs_ap=idx_sb[:, k * xch // 16:(k + 1) * xch // 16],
                    num_idxs=xch, num_idxs_reg=xch, elem_size=EE,
                    transpose=True, single_packet=False)

        # fwd needs chunk 0 first, bwd needs the last chunk first
        xorder = list(range(n_xc))
        if n_xc > 1:
            xorder = [xorder[0], xorder[-1]] + xorder[1:-1]
        issue_gather(xgc, enc_embed_bf, xidx_sb, xorder)

        eT = [gpool.tile([128, 2, NE], BF, name=f"eT{b}") for b in range(BPC)]
        gbT = [gpool.tile([128, 2, ND], BF, name=f"gbT{b}") for b in range(BPC)]
        for b in range(BPC):
            nc.gpsimd.dma_gather(
                out_ap=eT[b][:], in_ap=gembed_bf[:],
                idxs_ap=eidx_sb[:, b * NE // 16:(b + 1) * NE // 16],
                num_idxs=NE, num_idxs_reg=NE, elem_size=F, transpose=True)
            nc.gpsimd.dma_gather(
                out_ap=gbT[b][:], in_ap=w2t_bf[:],
                idxs_ap=gbidx_sb[:, b * ND // 16:(b + 1) * ND // 16],
                num_idxs=ND, num_idxs_reg=ND, elem_size=F, transpose=True)

        issue_gather(ygc, dec_embed_bf, yidx_sb, list(range(n_yc)))

        # persistent activation stores
        spool = es.enter_context(tc.tile_pool(name="stores", bufs=1))
        tcT = [spool.tile([128, 2, NE], BF, name=f"tcT{b}") for b in range(BPC)]
        lnZ = [spool.tile([1, NE], F32, name=f"lnZ{b}") for b in range(BPC)]
        hencTf = spool.tile([128, 2, BPC * NE], BF)
        hencTb = spool.tile([128, 2, BPC * NE], BF)
        hdecT = spool.tile([128, 2, BPC * (ND + 1)], BF)
        pout_sb = spool.tile([128, 16], F32)
        # t-major per-step h stores (contiguous writes); reshuffled to the
        # b-major layouts above just before the final phase
        hencFt = spool.tile([128, NE, 2, BPC], BF)
        hencBt = spool.tile([128, NE, 2, BPC], BF)
        hdecTt = spool.tile([128, ND, 2, BPC], BF)

        # ------------------------------------------------------------------
        # LSTM phase: H-major, weight-stationary.
        # PSUM banks (2KB each, padded): pg = g gate (rows 0:2), pfi = f+i
        # (rows 0:4), po = o (rows 0:2).  The x-part matmuls of step t+1 are
        # issued right after step t's h-matmuls so the PE stays busy during
        # the serial ACT/DVE tail.
        # ------------------------------------------------------------------
        lstm_sb = es.enter_context(tc.tile_pool(name="lstm_sb", bufs=2))

        BANK_NTS = (("g", ((0, 4), (1, 5))),
                    ("fi", ((0, 2), (1, 3), (2, 0), (3, 1))),
                    ("o", ((0, 6), (1, 7))))

        def lstm_phase(psp, W, n_steps, h0, ctg0, whh_sb, wih_sb,
                       x_slices_of, store_fn):
            PR = 2048 // (W * 4)

            def alloc_ps():
                return {bank: psp.tile([128, PR, W], F32, tag=f"p{bank}{W}",
                                       name=f"p{bank}")
                        for bank, _ in BANK_NTS}

            def x_mms(ps, t):
                for bank, rnts in BANK_NTS:
                    first = True
                    for row, nt in rnts:
                        for xt, c0, off, w in x_slices_of(t):
                            nc.tensor.matmul(
                                ps[bank][:, row, off:off + w],
                                wih_sb[:, nt, :], xt[:, 0, c0:c0 + w],
                                start=first, stop=False,
                                skip_group_check=True)
                            first = False

            def h_mms(ps, bank, rnts, h_prev):
                for row, nt in rnts:
                    nc.tensor.matmul(ps[bank][:, row, 0:W],
                                     whh_sb[:, nt * 2, :], h_prev[:, 0, 0:W],
                                     start=False, stop=False,
                                     skip_group_check=True)
                    nc.tensor.matmul(ps[bank][:, row, 0:W],
                                     whh_sb[:, nt * 2 + 1, :],
                                     h_prev[:, 1, 0:W],
                                     start=False, stop=True,
                                     skip_group_check=True)

            cur = alloc_ps()
            x_mms(cur, 0)
            h, ctg = h0, ctg0
            for t in range(n_steps):
                h_mms(cur, "g", BANK_NTS[0][1], h)
                # tanh(g) lands in the NEXT ctg tile rows 2:4 (rows 0:2 get
                # c_new below) so m01 is a single fused multiply
                ctg_n = lstm_sb.tile([128, 4, W], BF, tag=f"ctg{W}")
                nc.scalar.activation(ctg_n[:, 2:4, :], cur["g"][:, 0:2, :],
                                     AF.Tanh)
                h_mms(cur, "fi", BANK_NTS[1][1], h)
                sfi = lstm_sb.tile([128, 4, W], BF, tag=f"sfi{W}")
                nc.scalar.activation(sfi[:], cur["fi"][:, 0:4, :], AF.Sigmoid)
                h_mms(cur, "o", BANK_NTS[2][1], h)
                # m01 = [sf*c | si*tg]
                m01 = lstm_sb.tile([128, 4, W], BF, tag=f"m01{W}")
                nc.vector.tensor_mul(m01[:, 0:2, :], sfi[:, 0:2, :],
                                     ctg[:, 0:2, :])
                nc.vector.tensor_mul(m01[:, 2:4, :], sfi[:, 2:4, :],
                                     ctg_n[:, 2:4, :])
                nc.vector.tensor_add(ctg_n[:, 0:2, :], m01[:, 0:2, :],
                                     m01[:, 2:4, :])
                so = lstm_sb.tile([128, 2, W], BF, tag=f"so{W}")
                nc.scalar.activation(so[:], cur["o"][:, 0:2, :], AF.Sigmoid)
                tc_ = lstm_sb.tile([128, 2, W], BF, tag=f"tc{W}")
                nc.scalar.activation(tc_[:], ctg_n[:, 0:2, :], AF.Tanh)
                h_new = lstm_sb.tile([128, 2, W], BF, tag=f"h{W}", bufs=3)
                nc.vector.tensor_mul(h_new[:], so[:], tc_[:])
                store_fn(t, h_new)
                if t + 1 < n_steps:
                    nxt = alloc_ps()
                    x_mms(nxt, t + 1)
                    cur = nxt
                h, ctg = h_new, ctg_n
            return h, ctg

        # ------------------------------------------------------------------
        # Phase A: enc fwd + enc bwd, fused 64 moving columns
        # ------------------------------------------------------------------
        def xs_A(t):
            tb = n_enc - 1 - t
            return [(xgc[(t * B) // xch], (t * B) % xch, 0, 32),
                    (xgc[(tb * B) // xch], (tb * B) % xch, 32, 32)]

        def store_A(t, h):
            tb = n_enc - 1 - t
            nc.vector.tensor_copy(hencFt[:, t, :, :], h[:, :, 0:BPC])
            nc.vector.tensor_copy(hencBt[:, tb, :, :], h[:, :, 32:32 + BPC])

        with tc.tile_pool(name="psA", bufs=2, space="PSUM") as psA:
            ctg0 = lstm_sb.tile([128, 4, 64], BF, tag="ctg64")
            nc.vector.tensor_copy(ctg0[:, 0:2, :], cc0[:])
            h_fin, ctg_fin = lstm_phase(psA, 64, n_enc, hc0, ctg0,
                                        whh_e, wih_e, xs_A, store_A)
            nc.vector.tensor_copy(hdecT[:, :, 0::ND + 1][:, :, 0:BPC],
                                  h_fin[:, :, 0:BPC])

        # ------------------------------------------------------------------
        # Phase G (between the LSTM phases; its gathers ran during A)
        # ------------------------------------------------------------------
        with tc.tile_pool(name="gwork", bufs=2) as gw, \
             tc.tile_pool(name="gpsum", bufs=2, space="PSUM") as gp, \
             tc.tile_pool(name="zrow", bufs=4, space="PSUM") as zrp:
            # e = tanh(gembed[xs]) — emitted here (not at gather time) so it
            # does not block phase A's ACT stream behind the gpsimd gathers
            etan = [gpool.tile([128, 2, NE], BF, name=f"etan{b}")
                    for b in range(BPC)]
            for b in range(BPC):
                nc.scalar.activation(etan[b][:], eT[b][:], AF.Tanh)
            # conv + tanh
            for b in range(BPC):
                for fo in range(2):
                    cp = gp.tile([128, NE], F32, tag="convps")
                    first = True
                    for k in [2, 0, 1, 3, 4]:
                        d = k - 2
                        lo_out, lo_in = max(0, -d), max(0, d)
                        L = NE - abs(d)
                        for fi in range(2):
                            nc.tensor.matmul(
                                cp[:, lo_out:lo_out + L],
                                gconv_sb[:, (k * 2 + fi) * 2 + fo, :],
                                etan[b][:, fi, lo_in:lo_in + L],
                                start=first, stop=(k == 4 and fi == 1),
                                skip_group_check=True)
                            first = False
                    nc.scalar.activation(tcT[b][:, fo, :], cp[:], AF.Tanh)
            # logits (t-major) -> exp -> Z
            zrows = []
            for b in range(BPC):
                zrow = zrp.tile([1, NE], F32, tag="zrow", name=f"zr{b}")
                for ic in range(4):
                    zp = gw.tile([128, 4], F32, tag="zp")
                    for vc in range(4):
                        lp = gp.tile([128, 500], F32, tag="logps")
                        for f in range(2):
                            nc.tensor.matmul(
                                lp[:], tcT[b][:, f, ic * 128:(ic + 1) * 128],
                                w2_sb[:, f, vc * 500:(vc + 1) * 500],
                                start=(f == 0), stop=(f == 1))
                        sc = gw.tile([128, 500], BF, tag="expsc")
                        nc.scalar.activation(sc[:], lp[:], AF.Exp,
                                             accum_out=zp[:, vc:vc + 1])
                    zc = gw.tile([128, 1], F32, tag="zc")
                    nc.vector.tensor_reduce(zc[:], zp[:],
                                            axis=mybir.AxisListType.X,
                                            op=mybir.AluOpType.add)
                    nc.tensor.transpose(zrow[:, ic * 128:(ic + 1) * 128],
                                        zc[:], idf32[:])
                zrows.append(zrow)
            for b in range(BPC):
                nc.scalar.activation(lnZ[b][:], zrows[b][:], AF.Ln)

        # ------------------------------------------------------------------
        # Phase B: decoder
        # ------------------------------------------------------------------
        def xs_B(t):
            return [(ygc[(t * B) // xch], (t * B) % xch, 0, 32)]

        def store_B(t, h):
            nc.vector.tensor_copy(hdecTt[:, t, :, :], h[:, :, 0:BPC])

        with tc.tile_pool(name="psB", bufs=2, space="PSUM") as psB:
            ctg0B = lstm_sb.tile([128, 4, 32], BF, tag="ctg32")
            nc.vector.tensor_copy(ctg0B[:, 0:2, :], ctg_fin[:, 0:2, 0:32])
            lstm_phase(psB, 32, n_dec, h_fin, ctg0B,
                       whh_dd, wih_dd, xs_B, store_B)

        # ------------------------------------------------------------------
        # Final phase
        # ------------------------------------------------------------------
        # reshuffle the t-major step stores into b-major contiguous layouts
        for hc in range(2):
            for b in range(BPC):
                nc.vector.tensor_copy(hencTf[:, hc, b * NE:(b + 1) * NE],
                                      hencFt[:, :, hc, b])
                nc.vector.tensor_copy(hencTb[:, hc, b * NE:(b + 1) * NE],
                                      hencBt[:, :, hc, b])
                o = b * (ND + 1) + 1
                nc.vector.tensor_copy(hdecT[:, hc, o:o + ND],
                                      hdecTt[:, :, hc, b])

        with tc.tile_pool(name="fin_sb", bufs=2) as fsb, \
             tc.tile_pool(name="fin_keep", bufs=1) as fkeep, \
             tc.tile_pool(name="fin_ps", bufs=2, space="PSUM") as fps:
            sda = [fkeep.tile([128, 8], F32, name=f"sda{b}")
                   for b in range(BPC)]
            for b in range(BPC):
                thT = fsb.tile([128, 2, NE], BF, tag="thT")
                for hc in range(2):
                    tp = fps.tile([128, NE], F32, tag="thps")
                    for ec in range(4):
                        src = hencTf if ec < 2 else hencTb
                        nc.tensor.matmul(
                            tp[:], tt_sb[:, ec * 2 + hc, :],
                            src[:, ec % 2, b * NE:(b + 1) * NE],
                            start=(ec == 0), stop=(ec == 3))
                    nc.scalar.activation(thT[:, hc, :], tp[:], AF.Copy)
                for jc in range(4):
                    fp = fps.tile([128, NE], F32, tag="fps")
                    for hc in range(2):
                        nc.tensor.matmul(
                            fp[:],
                            hdecT[:, hc, :][:, b * (ND + 1) + jc * 128:
                                            b * (ND + 1) + jc * 128 + 128],
                            thT[:, hc, :], start=(hc == 0), stop=False,
                            skip_group_check=True)
                    sc1 = fsb.tile([128, NE], BF, tag="fexp")
                    nc.scalar.activation(
                        sc1[:], fp[:], AF.Exp,
                        accum_out=sda[b][:, 2 * jc:2 * jc + 1])
                    for f in range(2):
                        nc.tensor.matmul(
                            fp[:], gbT[b][:, f, jc * 128:jc * 128 + 128],
                            tcT[b][:, f, :], start=False, stop=False,
                            skip_group_check=True)
                    nc.tensor.matmul(fp[:], negones[:, 0:128], lnZ[b][:],
                                     start=False, stop=True,
                                     skip_group_check=True)
                    sc2 = fsb.tile([128, NE], BF, tag="fexp")
                    nc.scalar.activation(
                        sc2[:], fp[:], AF.Exp,
                        accum_out=sda[b][:, 2 * jc + 1:2 * jc + 2])
            for b in range(BPC):
                lns = fsb.tile([128, 8], F32, tag="lns")
                nc.scalar.activation(lns[:], sda[b][:], AF.Ln)
                for jc in range(4):
                    nc.vector.tensor_sub(
                        pout_sb[:, b * 4 + jc:b * 4 + jc + 1],
                        lns[:, 2 * jc + 1:2 * jc + 2],
                        lns[:, 2 * jc:2 * jc + 1])
            nc.sync.dma_start(out=pout[:], in_=pout_sb[:])

    nc.compile()
    return nc


# ---------------------------------------------------------------------------
# host side
# ---------------------------------------------------------------------------

_CACHE = {}


def _get_program(n_enc, n_dec):
    key = (n_enc, n_dec)
    if key not in _CACHE:
        _CACHE[key] = build_program(n_enc, n_dec)
    return _CACHE[key]


def _host_prep(inputs, n_enc=NE, n_dec=ND):
    xs = np.asarray(inputs["xs_idx"]).astype(np.int64)
    ys = np.asarray(inputs["ys_idx"]).astype(np.int64)
    gembed_W = np.asarray(inputs["gembed_W"], np.float32)
    gconv_W = np.asarray(inputs["gconv_W"], np.float32)
    gdecode_W = np.asarray(inputs["gdecode_W"], np.float32)
    enc_embed = np.asarray(inputs["enc_embed"], np.float32)
    dec_embed = np.asarray(inputs["dec_embed"], np.float32)
    T = np.asarray(inputs["T"], np.float32)

    for nm in ("enc_b", "dec_b"):
        assert not np.any(np.asarray(inputs[nm])), f"{nm} nonzero unsupported"

    def lstm_w(wih, whh):
        wih = np.asarray(wih, np.float32)  # (4H, E)
        whh = np.asarray(whh, np.float32)  # (4H, H)
        wih_t = _bf(wih.T.reshape(128, 8, 128))
        whh_t = _bf(whh.T.reshape(2, 128, 8, 128)
                    .transpose(1, 2, 0, 3).reshape(128, 16, 128))
        return wih_t, whh_t

    wih_e_d, whh_e_d = lstm_w(inputs["enc_Wih"], inputs["enc_Whh"])
    wih_d_d, whh_d_d = lstm_w(inputs["dec_Wih"], inputs["dec_Whh"])

    w2_d = _bf(gdecode_W.reshape(2, 128, V).transpose(1, 0, 2))
    g = gconv_W.reshape(KW, 2, 128, 2, 128)
    gconv_d = _bf(np.ascontiguousarray(
        g.transpose(2, 0, 1, 3, 4).reshape(128, KW * 4, 128)))
    tt = T.T.reshape(4, 128, 2, 128)  # [ec, p, hc, c]
    tt_d = _bf(np.ascontiguousarray(
        tt.transpose(1, 0, 2, 3).reshape(128, 8, 128)))

    base = dict(
        gembed_bf=_bf(gembed_W), enc_embed_bf=_bf(enc_embed),
        dec_embed_bf=_bf(dec_embed), w2t_bf=_bf(gdecode_W.T),
        w2_d=w2_d, gconv_d=gconv_d,
        wih_e_d=wih_e_d, whh_e_d=whh_e_d,
        wih_d_d=wih_d_d, whh_d_d=whh_d_d, tt_d=tt_d,
    )

    in_maps = []
    for m in range(NCORES):
        order = np.concatenate(
            [np.arange(4 * m, 4 * m + 4),
             np.delete(np.arange(B), np.s_[4 * m:4 * m + 4])])
        xs_p, ys_p = xs[order], ys[order]
        xm = np.where(xs_p < PG, 0, xs_p)
        ym = np.where(ys_p < PG, 0, ys_p)
        im = dict(base)
        im["x_enc_idx"] = _wrap16(xm[:, :n_enc].T)   # (t,b) order
        im["y_dec_idx"] = _wrap16(ym[:, :n_dec].T)
        im["e_idx"] = _wrap16(xs_p[:BPC])            # (b,t) order
        im["gb_idx"] = _wrap16(ys_p[:BPC])
        in_maps.append(im)
    return in_maps


def kernel(**inputs):
    trace = bool(int(os.environ.get("KERNEL_TRACE", "0")))
    n_enc = int(os.environ.get("KERNEL_NENC", NE))
    n_dec = int(os.environ.get("KERNEL_NDEC", ND))
    nc = _get_program(n_enc, n_dec)
    in_maps = _host_prep(inputs, n_enc, n_dec)
    res = run_bass_kernel_spmd(nc, in_maps, list(range(NCORES)), trace=trace)
    total = np.float64(0.0)
    for r in res.results:
        total += np.asarray(r["pout"], np.float64).sum()
    kernel.last_results = res
    return np.float32(-total)
